# revision 67
# baseline (speedup 1.0000x reference)
"""Causal self-attention (B=4, T=2048, C=1024, H=16, D=64) on 8 TRN2 NeuronCores.

Sharding: 4 batches x 2 head-groups (8 heads each). Core c handles batch c//2,
heads 8*(c%2) .. 8*(c%2)+7. Host pre-transposes x and slices/transposes the
weights so the device kernel needs no on-chip transposes. All matmul operands
are bf16 (fp32 PSUM accumulation).

Fused single-pass schedule: the QKV projection (4 t-chunks x 12 matmul groups)
and the attention/proj work are interleaved in one stream. Attention for query
chunk qc starts as soon as projection chunk tci=qc is done; remaining
projection groups are rationed into the attention loop as PE filler so the
scalar engine (exp) and PE stay busy together instead of running as separate
phases.

  projection: qkT[feat, T] = Wqk_g @ x^T, V[t, vfeat] = x @ Wv_g^T (+ ones col)
  attention:  per head-pair (2p, 2p+1) and query chunk qc: for each k-block j,
              S^T blocks for BOTH heads land in one [128,2,512] PSUM tile via
              row-tiled matmuls (head 2p on PE tile (0,0) from SBUF partitions
              0:64, head 2p+1 on (64,0) — concurrent); ONE exp call covers the
              pair, so both heads' next scores are gated by the same semaphore
              and issue back-to-back (keeps the pairing aligned).
              P^T = exp(S^T/8) * causal masks, out^T[d|sum, q] = [V_h|1]^T P^T,
              normalized by approx-reciprocal + gpsimd partition-broadcast.
  proj:       y = attn^T.T @ Wp_g^T, interleaved between pairs as PE filler

Each core returns a [2048, 1024] partial; the host sums the two head-group
partials per batch.
"""

import numpy as np

T = 2048
N_CORES = 8

_CACHE = {}


def _build_module():
    from collections import deque
    from contextlib import ExitStack

    import concourse.tile as tile
    from concourse.tile_rust import add_dep_helper
    from concourse import bacc, mybir

    f32 = mybir.dt.float32
    bf16 = mybir.dt.bfloat16
    Exp = mybir.ActivationFunctionType.Exp
    Copy = mybir.ActivationFunctionType.Copy

    nc = bacc.Bacc("TRN2", target_bir_lowering=False, debug=False,
                   num_devices=N_CORES)

    xT_d = nc.dram_tensor("xT", (1024, 2048), bf16, kind="ExternalInput").ap()
    wqkT_d = nc.dram_tensor("wqkT", (1024, 1024), bf16, kind="ExternalInput").ap()
    wvT_d = nc.dram_tensor("wvT", (1024, 512), bf16, kind="ExternalInput").ap()
    wpT_d = nc.dram_tensor("wpT", (512, 1024), bf16, kind="ExternalInput").ap()
    mk_d = nc.dram_tensor("trimask", (128, 128), bf16, kind="ExternalInput").ap()
    y_d = nc.dram_tensor("y", (2048, 1024), bf16, kind="ExternalOutput").ap()

    with tile.TileContext(nc) as tc, ExitStack() as ctx:
        pers = ctx.enter_context(tc.tile_pool(name="pers", bufs=1))
        sb_qT = pers.tile([128, 4, 2048], bf16, name="sb_qT")
        sb_kT = pers.tile([128, 4, 2048], bf16, name="sb_kT")
        sb_v = pers.tile([128, 16, 520], bf16, name="sb_v")
        v_view = sb_v[:].rearrange("p t (h e) -> p t h e", e=65)
        sb_attnT = pers.tile([128, 4, 2048], bf16, name="sb_attnT")
        sb_wpT = pers.tile([128, 4, 1024], bf16, name="sb_wpT")
        sb_mask2 = pers.tile([128, 2, 128], bf16, name="sb_mask2")
        sb_wvT = pers.tile([128, 8, 512], bf16, name="sb_wvT")

        wqk_pool = ctx.enter_context(tc.tile_pool(name="wqk", bufs=8))
        xt_pool = ctx.enter_context(tc.tile_pool(name="xt", bufs=4))
        ps_misc = ctx.enter_context(tc.tile_pool(name="ps_misc", bufs=2,
                                                 space="PSUM"))
        ps_s = ctx.enter_context(tc.tile_pool(name="ps_s", bufs=2, space="PSUM"))
        ps_o = ctx.enter_context(tc.tile_pool(name="ps_o", bufs=2, space="PSUM"))
        exp_pool = ctx.enter_context(tc.tile_pool(name="expp", bufs=6))
        norm_pool = ctx.enter_context(tc.tile_pool(name="normp", bufs=3))
        y_pool = ctx.enter_context(tc.tile_pool(name="yp", bufs=3))
        misc_sb = ctx.enter_context(tc.tile_pool(name="miscsb", bufs=1))

        wqkT_r = wqkT_d.rearrange("(co ci) f -> ci co f", ci=128)
        xT_r = xT_d.rearrange("(co ci) t -> ci co t", ci=128)
        wvT_r = wvT_d.rearrange("(co ci) f -> ci co f", ci=128)
        wpT_r = wpT_d.rearrange("(ko ki) n -> ki ko n", ki=128)

        # ---- PE warmup: dummy matmuls on zeroed SBUF fill the initial DMA
        # wait so the HAM clock gate reaches K=8/8 (2.4 GHz) before the first
        # real matmul; otherwise the first ~3.4us of real work runs at 1.2 GHz.
        wz = misc_sb.tile([128, 512], bf16, name="wz")
        nc.vector.memset(wz[:], 0.0)

        # single scratch psum tile reused by every dummy matmul (never read;
        # a fresh ring tile per dummy would alias the accumulating groups)
        psd = ps_misc.tile([128, 512], f32, tag="psb", name="warm")

        def warm_pe(n):
            # dummy matmuls on zeroed SBUF keep the HAM clock gate at K=8/8
            # (2.4 GHz) while the PE waits on DMA
            for _ in range(n):
                nc.tensor.matmul(psd[:], lhsT=wz[:, 0:128], rhs=wz[:],
                                 start=True, stop=True)

        warm_pe(11)

        # ---- DMA admission. The sync-engine HWDGE queue executes DMAs in
        # emission order, so order = priority. x1 goes on the scalar
        # engine's HWDGE queue, which runs in parallel with the sync queue
        # (baseline: x1/x2 arrived ~40/60us causing multi-us PE stalls and
        # HAM re-throttles). x2/x3 ride the sync queue after the weights.
        fbs = [4, 0, 5, 1, 6, 2, 7, 3]
        wtiles = {}
        wt = wqk_pool.tile([128, 8, 128], bf16, tag="wqk", name="wt4")
        nc.sync.dma_start(wt[:], wqkT_r[:, :, 4 * 128:5 * 128])
        wtiles[4] = wt
        xchunks = {}
        for tci in range(4):
            xchunks[tci] = xt_pool.tile([128, 8, 512], bf16, tag="xt",
                                        name=f"xc{tci}")
        # x0 split across BOTH queues (even co on sync, odd co on scalar):
        # per-co descriptors for fan-out, two queues for ~2x aggregate
        # bandwidth — all of x0 gates the qT/kT groups and first scores
        xdmas = []
        for co in range(8):
            eng = nc.sync if co % 2 == 0 else nc.scalar
            xdmas.append(nc.sync.dma_start(xchunks[0][:, co, :],
                                           xT_r[:, co, 0:512])
                         if co % 2 == 0 else
                         nc.scalar.dma_start(xchunks[0][:, co, :],
                                             xT_r[:, co, 0:512]))
        prev = xdmas[-2]

        # scalar-engine queue continues: wvT then x1 (in-order behind x0's
        # odd chunks, so they don't steal HBM bandwidth from x0; wvT is
        # needed ~17us in — far too late when queued on sync behind the
        # weights)
        for m in range(2):
            nc.scalar.dma_start(sb_mask2[:, m, :], mk_d[:])

        def _stage_w(fb, prev):
            wt = wqk_pool.tile([128, 8, 128], bf16, tag="wqk", name=f"wt{fb}")
            d = nc.sync.dma_start(wt[:], wqkT_r[:, :, fb * 128:(fb + 1) * 128])
            add_dep_helper(d.ins, prev.ins, sync=False, reason="stage w")
            wtiles[fb] = wt
            return d

        prev = _stage_w(0, prev)
        for fb in (5, 1, 6, 2, 7, 3):
            prev = _stage_w(fb, prev)
            if fb == 5:
                # wvT held until wt5 lands: it contends with the wt5/wt1
                # deadline (~16us) otherwise; needed itself at ~19us
                wv_dma = nc.scalar.dma_start(sb_wvT[:], wvT_r[:])
                add_dep_helper(wv_dma.ins, prev.ins, sync=True,
                               reason="wv after wt5")
        wv_last = prev

        # ones column of sb_v via exp(0)=1 — also warms the ACT exp table
        zeros = misc_sb.tile([128, 128], f32, name="zeros")
        nc.vector.memset(zeros[:], 0.0)
        ones_row = misc_sb.tile([1, 128], bf16, name="ones_row")
        nc.vector.memset(ones_row[:], 1.0)
        nc.scalar.activation(
            v_view[:, :, :, 64:65],
            zeros[:].rearrange("p (a b c) -> p a b c", a=16, b=8),
            Exp,
        )

        # ---- projection group emitters ----
        round_copy = {}

        def qk_group(tci, fb, pad=False):
            dst, pblk = (sb_kT, fb - 4) if fb >= 4 else (sb_qT, fb)
            ps = ps_misc.tile([128, 512], f32, tag="psb", name="psqk")
            for co in range(8):
                nc.tensor.matmul(
                    ps[:],
                    lhsT=wtiles[fb][:, co, :],
                    rhs=xchunks[tci][:, co, :],
                    start=(co == 0), stop=(co == 7),
                )
                if pad and co < 7:
                    # dummy between co-steps: x0 arrives per-co slower than
                    # the matmuls consume it; keeps HAM warm through the
                    # trickle instead of oscillating to K=4/8
                    warm_pe(1)
            cp = nc.vector.tensor_copy(
                dst[:, pblk, tci * 512:(tci + 1) * 512], ps[:])
            if fb == 4:
                round_copy[tci] = cp

        def v_group(tci, tb):
            tblk = tci * 4 + tb
            ps = ps_misc.tile([128, 512], f32, tag="psb", name="psv")
            for co in range(8):
                nc.tensor.matmul(
                    ps[:],
                    lhsT=xchunks[tci][:, co, tb * 128:(tb + 1) * 128],
                    rhs=sb_wvT[:, co, :],
                    start=(co == 0), stop=(co == 7),
                )
            nc.vector.tensor_copy(
                v_view[:, tblk, :, 0:64],
                ps[:].rearrange("p (h d) -> p h d", d=64),
            )

        def groups_for(tci):
            gs = []
            for fb in fbs:
                gs.append((tci, lambda t=tci, f=fb: qk_group(t, f)))
            for tb in range(4):
                gs.append((tci, lambda t=tci, b=tb: v_group(t, b)))
            return gs

        # ---- attention emitters ----
        def emit_proj_half(tblk, n, on_act=False):
            ysb = y_pool.tile([128, 512], bf16, tag="ysb", name="ysb")
            pj = ps_misc.tile([128, 512], f32, tag="psb", name="pj")
            for ko in range(4):
                nc.tensor.matmul(
                    pj[:],
                    lhsT=sb_attnT[:, ko, tblk * 128:(tblk + 1) * 128],
                    rhs=sb_wpT[:, ko, n * 512:(n + 1) * 512],
                    start=(ko == 0), stop=(ko == 3),
                )
            # tail projs alternate ACT/DVE so neither engine's copy queue
            # paces the PE
            if on_act and n == 0:
                nc.scalar.activation(ysb[:], pj[:], Copy)
            else:
                nc.vector.tensor_copy(ysb[:], pj[:])
            nc.sync.dma_start(
                y_d[tblk * 128:(tblk + 1) * 128, n * 512:(n + 1) * 512],
                ysb[:])

        def emit_proj(tblk, on_act=False):
            for n in range(2):
                emit_proj_half(tblk, n, on_act=on_act)

        def norm_store(po, rr, p_, qc, on_act=False, on_pe=False):
            att_slice = sb_attnT[rr:rr + 64, p_, qc * 512:(qc + 1) * 512]
            if on_pe:
                # tail path: head B's copies on ACT in parallel with head
                # A's DVE chain; head A's multiply reads its po straight
                # from PSUM (no att copy — same partitions, rr=0); bf16
                # broadcast on the now-idle PE (the gpsimd broadcast is
                # ~1-2us; fp32 matmul is a slow LOW/HIGH double pass).
                sums = norm_pool.tile([1, 512], f32, tag="sums", name="sums")
                if on_act:
                    nc.scalar.activation(att_slice, po[0:64, :], Copy)
                    nc.scalar.activation(sums[:], po[64:65, :], Copy)
                else:
                    nc.vector.tensor_copy(att_slice, po[0:64, :])
                    nc.vector.tensor_copy(sums[:], po[64:65, :])
                recip = norm_pool.tile([1, 512], f32, tag="recip",
                                       name="recip")
                nc.vector.reciprocal_approx_fast(out=recip[:], in_=sums[:])
                recipb = norm_pool.tile([1, 512], bf16, tag="recipb",
                                        name="recipb")
                nc.vector.tensor_copy(recipb[:], recip[:])
                # bc from ps_misc: a ps_o tile here would cycle into po's
                # slot and deadlock (bc waits po's readers; the mul reading
                # po would wait bc)
                bc = ps_misc.tile([128, 512], f32, tag="psb", name="bcps")
                nc.tensor.matmul(bc[:], lhsT=ones_row[:], rhs=recipb[:],
                                 start=True, stop=True)
                nc.vector.tensor_mul(att_slice, att_slice, bc[rr:rr + 64, :])
                return None
            sums = norm_pool.tile([1, 512], f32, tag="sums", name="sums")
            if on_act:
                nc.scalar.activation(att_slice, po[0:64, :], Copy)
                nc.scalar.activation(sums[:], po[64:65, :], Copy)
            else:
                nc.vector.tensor_copy(att_slice, po[0:64, :])
                nc.vector.tensor_copy(sums[:], po[64:65, :])
            recip = norm_pool.tile([1, 512], f32, tag="recip", name="recip")
            nc.vector.reciprocal_approx_fast(out=recip[:], in_=sums[:])
            # The broadcast-gated multiply is returned as a deferred
            # closure: emitted into the DVE FIFO only after the next
            # duo's filler casts, so its gpsimd-wait can't head-of-line
            # block the casts that free the fillers' PSUM slots.
            bcast = norm_pool.tile([128, 512], f32, tag="bcast",
                                   name="bcast")
            nc.gpsimd.partition_broadcast(bcast[:], recip[:])

            def _mul():
                nc.vector.tensor_mul(att_slice, att_slice,
                                     bcast[rr:rr + 64, :])
            return _mul

        def attn_duo(qc, di):
            """Generator: yields at PE-filler points."""
            hA, hB = 2 * di, 2 * di + 1
            nblk = 4 * qc + 4
            poA = ps_o.tile([65, 512], f32, tag="pso", name="poA")
            poB = ps_o.tile([65, 512], f32, tag="pso", name="poB")
            ets = {}

            def emit_pv(j):
                et = ets.pop(j)
                lo = max(0, (j - 4 * qc)) * 128
                for idx, (h, po) in enumerate(((hA, poA), (hB, poB))):
                    nc.tensor.matmul(
                        po[:, lo:512],
                        lhsT=v_view[:, j, h, :],
                        rhs=et[:, idx, lo:512],
                        start=(j == 0), stop=(j == nblk - 1),
                    )

            for j in range(nblk):
                lo = max(0, (j - 4 * qc)) * 128
                pss = ps_s.tile([128, 2, 512], f32, tag="pss", name="pss")
                for idx, rr in enumerate((0, 64)):
                    nc.tensor.matmul(
                        pss[:, idx, lo:512],
                        lhsT=sb_kT[rr:rr + 64, di, j * 128:(j + 1) * 128],
                        rhs=sb_qT[rr:rr + 64, di,
                                  qc * 512 + lo:(qc + 1) * 512],
                        start=True, stop=True,
                        tile_position=(rr, 0),
                    )
                et = exp_pool.tile([128, 2, 512], bf16, tag="expT", name="et")
                nc.scalar.activation(et[:, :, lo:512], pss[:, :, lo:512],
                                     Exp, scale=0.125)
                if j >= 4 * qc:
                    nc.vector.tensor_mul(et[:, :, lo:lo + 128],
                                         et[:, :, lo:lo + 128], sb_mask2[:])
                ets[j] = et
                if j >= 1:
                    yield
                    emit_pv(j - 1)
                else:
                    yield
            emit_pv(nblk - 1)
            last = qc == 3 and di == 3
            if last:
                # keep the HAM clock gate warm through the final norm's DVE
                # chain so the tail projections run at 2.4 GHz. Dedicated
                # tail-local psum tile: writing the head's psd here would
                # add a backward dep poisoning every ps_misc slot reuse.
                psd2 = ps_o.tile([128, 512], f32, tag="pso", name="warmtail")
                for _ in range(6):
                    nc.tensor.matmul(psd2[:], lhsT=wz[:, 0:128],
                                     rhs=et[:, 0, :], start=True, stop=True)
            # early qc: ACT has slack (small exp load), so po copies go
            # there; late qc is ACT-bound so they stay on DVE
            for m in (norm_store(poA, 0, di, qc, on_act=(qc <= 1),
                                 on_pe=last),
                      norm_store(poB, 64, di, qc, on_act=(qc <= 1) or last,
                                 on_pe=last)):
                if m is not None:
                    pend_muls.append(m)
            yield

        # ---- fused schedule ----
        # wpT / mask admission chained behind round-2 projection traffic
        def admit_late():
            prev = wv_last
            dma = nc.sync.dma_start(xchunks[1][:], xT_r[:, :, 512:1024])
            add_dep_helper(dma.ins, prev.ins, sync=False,
                           reason="stage x1 after weights")
            prev = dma
            dma = nc.sync.dma_start(sb_wpT[:], wpT_r[:])
            add_dep_helper(dma.ins, prev.ins, sync=False,
                           reason="admit wpT after weights")
            prev = dma
            for tci in (2, 3):
                dma = nc.sync.dma_start(
                    xchunks[tci][:], xT_r[:, :, tci * 512:(tci + 1) * 512])
                add_dep_helper(dma.ins, prev.ins, sync=False,
                               reason="stage late x")
                prev = dma

        # tci0 minimal prefix: exactly what attention (qc0, duo0) needs —
        # kT pair 0 (fb4), qT pair 0 (fb0). The remaining tci0 groups go to
        # the filler queue so duo d's needs (fb 4+d, fb d) and the v blocks
        # cascade in as earlier duos run (matching DMA arrival order).
        qk_group(0, 4, pad=True)
        qk_group(0, 0)
        admit_late()

        # Just-in-time fillers: projection group tci=k is emitted inside the
        # attention stretch qc=k, whose exp load it naturally balances
        # (PE attn(qc)+groups(tci=qc) ~ ACT exp(qc) for every qc). Each duo
        # pulls its own q/k feature blocks at start, prefetches the next
        # duo's mid-unit, and the first duo of each qc pulls the v blocks.
        gq = {}
        gv = {}
        for tci in range(4):
            for fb in fbs:
                if not (tci == 0 and fb in (4, 0)):
                    gq[(tci, fb)] = (lambda t=tci, f=fb: qk_group(t, f))
            for tb in range(4):
                gv[(tci, tb)] = (lambda t=tci, b=tb: v_group(t, b))

        def pop_pair(qc, di):
            if qc > 3:
                return
            for fb in (di, 4 + di):
                g = gq.pop((qc, fb), None)
                if g:
                    g()

        def pop_v(qc, tb):
            g = gv.pop((qc, tb), None)
            if g:
                g()

        pend_muls = []

        def flush_muls():
            for m in pend_muls:
                m()
            pend_muls.clear()

        # ---- qc=0: DMA-arrival-paced pops (delicate head ordering) ----
        qc = 0
        for di in range(4):
            pop_pair(qc, di)
            tick = 0
            for _ in attn_duo(qc, di):
                tick += 1
                if di == 0 and tick <= 4:
                    pop_v(qc, tick - 1)
                if tick == 1 and di < 3:
                    pop_pair(qc, di + 1)
                if tick == 2 and pend_muls:
                    flush_muls()

        # ---- qc>=1: one evenly-paced work queue per stretch. v units
        # first (PV j=4qc+tb inside duo di=0 consumes them), then qk and
        # proj halves interleaved; one unit per tick keeps the PE fed
        # through the diagonal (narrow) j-blocks where attention alone
        # can't cover the exp latency. ----
        for qc in range(1, 4):
            vs = [gv.pop((qc, tb)) for tb in range(4)]
            qks = [gq.pop((qc, fb)) for fb in fbs if (qc, fb) in gq]
            projs = [(lambda b=(qc - 1) * 4 + t, nn=n:
                      emit_proj_half(b, nn))
                     for t in range(4) for n in range(2)]
            rest = [u for pair in zip(qks, projs) for u in pair]
            rest += qks[len(projs):] + projs[len(qks):]
            total_ticks = 4 * (4 * qc + 5)
            rest_ticks = total_ticks - 4
            popped = 0
            tg = 0
            for di in range(4):
                tick = 0
                for _ in attn_duo(qc, di):
                    tg += 1
                    tick += 1
                    if tick == 2 and pend_muls:
                        flush_muls()
                    if tg <= 4:
                        vs[tg - 1]()
                    elif popped < len(rest) and (
                            (tg - 4) * len(rest) >= (popped + 1) * rest_ticks
                            or len(rest) - popped >= total_ticks - tg):
                        rest[popped]()
                        popped += 1
            while popped < len(rest):
                rest[popped]()
                popped += 1
        flush_muls()
        for tblk in range(12, 16):
            emit_proj(tblk, on_act=True)

    nc.compile()
    return nc


def _get_module():
    if "nc" not in _CACHE:
        _CACHE["nc"] = _build_module()
    return _CACHE["nc"]


def _make_trimask():
    # trimask[kk, q] = 1 iff q >= kk (diagonal 128x128 block)
    q = np.arange(128)[None, :]
    kk = np.arange(128)[:, None]
    return (q >= kk).astype(np.float32)


def make_in_maps(x, W_qkv, W_proj):
    import ml_dtypes

    bf16 = ml_dtypes.bfloat16
    x = np.asarray(x, dtype=np.float32)
    W_qkv = np.asarray(W_qkv, dtype=np.float32)
    W_proj = np.asarray(W_proj, dtype=np.float32)
    trimask = _make_trimask().astype(bf16)
    in_maps = []
    for c in range(N_CORES):
        b, g = c // 2, c % 2
        s = 512 * g
        wqk = np.concatenate([W_qkv[s:s + 512], W_qkv[1024 + s:1024 + s + 512]], 0)
        in_maps.append({
            "xT": np.ascontiguousarray(x[b].T).astype(bf16),
            "wqkT": np.ascontiguousarray(wqk.T).astype(bf16),
            "wvT": np.ascontiguousarray(W_qkv[2048 + s:2048 + s + 512].T).astype(bf16),
            "wpT": np.ascontiguousarray(W_proj[:, s:s + 512].T).astype(bf16),
            "trimask": trimask,
        })
    return in_maps


def run(x, W_qkv, W_proj, trace=False):
    """Returns (y_full [4,2048,1024], BassKernelResults)."""
    from concourse import bass_utils

    nc = _get_module()
    in_maps = make_in_maps(x, W_qkv, W_proj)
    res = bass_utils.run_bass_kernel_spmd(
        nc, in_maps, core_ids=list(range(N_CORES)), trace=trace)
    y = np.zeros((4, T, 1024), np.float32)
    for b in range(4):
        y[b] = (res.results[2 * b]["y"].astype(np.float32)
                + res.results[2 * b + 1]["y"].astype(np.float32))
    return y, res


def kernel(x, W_qkv, W_proj):
    y, _ = run(x, W_qkv, W_proj, trace=False)
    return y



# revision 70
# speedup vs baseline: 1.0321x; 1.0321x over previous
"""Causal self-attention (B=4, T=2048, C=1024, H=16, D=64) on 8 TRN2 NeuronCores.

Sharding: 4 batches x 2 head-groups (8 heads each). Core c handles batch c//2,
heads 8*(c%2) .. 8*(c%2)+7. Host pre-transposes x and slices/transposes the
weights so the device kernel needs no on-chip transposes. All matmul operands
are bf16 (fp32 PSUM accumulation).

Fused single-pass schedule: the QKV projection (4 t-chunks x 12 matmul groups)
and the attention/proj work are interleaved in one stream. Attention for query
chunk qc starts as soon as projection chunk tci=qc is done; remaining
projection groups are rationed into the attention loop as PE filler so the
scalar engine (exp) and PE stay busy together instead of running as separate
phases.

  projection: qkT[feat, T] = Wqk_g @ x^T, V[t, vfeat] = x @ Wv_g^T (+ ones col)
  attention:  per head-pair (2p, 2p+1) and query chunk qc: for each k-block j,
              S^T blocks for BOTH heads land in one [128,2,512] PSUM tile via
              row-tiled matmuls (head 2p on PE tile (0,0) from SBUF partitions
              0:64, head 2p+1 on (64,0) — concurrent); ONE exp call covers the
              pair, so both heads' next scores are gated by the same semaphore
              and issue back-to-back (keeps the pairing aligned).
              P^T = exp(S^T/8) * causal masks, out^T[d|sum, q] = [V_h|1]^T P^T,
              normalized by approx-reciprocal + gpsimd partition-broadcast.
  proj:       y = attn^T.T @ Wp_g^T, interleaved between pairs as PE filler

Each core returns a [2048, 1024] partial; the host sums the two head-group
partials per batch.
"""

import numpy as np

T = 2048
N_CORES = 8

_CACHE = {}


def _build_module():
    from collections import deque
    from contextlib import ExitStack

    import concourse.tile as tile
    from concourse.tile_rust import add_dep_helper
    from concourse import bacc, mybir

    f32 = mybir.dt.float32
    bf16 = mybir.dt.bfloat16
    Exp = mybir.ActivationFunctionType.Exp
    Copy = mybir.ActivationFunctionType.Copy

    nc = bacc.Bacc("TRN2", target_bir_lowering=False, debug=False,
                   num_devices=N_CORES)

    xT_d = nc.dram_tensor("xT", (1024, 2048), bf16, kind="ExternalInput").ap()
    wqkT_d = nc.dram_tensor("wqkT", (1024, 1024), bf16, kind="ExternalInput").ap()
    wvT_d = nc.dram_tensor("wvT", (1024, 512), bf16, kind="ExternalInput").ap()
    wpT_d = nc.dram_tensor("wpT", (512, 1024), bf16, kind="ExternalInput").ap()
    mk_d = nc.dram_tensor("trimask", (128, 128), bf16, kind="ExternalInput").ap()
    y_d = nc.dram_tensor("y", (2048, 1024), bf16, kind="ExternalOutput").ap()

    with tile.TileContext(nc) as tc, ExitStack() as ctx:
        pers = ctx.enter_context(tc.tile_pool(name="pers", bufs=1))
        sb_qT = pers.tile([128, 4, 2048], bf16, name="sb_qT")
        sb_kT = pers.tile([128, 4, 2048], bf16, name="sb_kT")
        sb_v = pers.tile([128, 16, 520], bf16, name="sb_v")
        v_view = sb_v[:].rearrange("p t (h e) -> p t h e", e=65)
        sb_attnT = pers.tile([128, 4, 2048], bf16, name="sb_attnT")
        sb_wpT = pers.tile([128, 4, 1024], bf16, name="sb_wpT")
        sb_mask2 = pers.tile([128, 2, 128], bf16, name="sb_mask2")
        sb_wvT = pers.tile([128, 8, 512], bf16, name="sb_wvT")

        wqk_pool = ctx.enter_context(tc.tile_pool(name="wqk", bufs=8))
        xt_pool = ctx.enter_context(tc.tile_pool(name="xt", bufs=4))
        ps_misc = ctx.enter_context(tc.tile_pool(name="ps_misc", bufs=2,
                                                 space="PSUM"))
        ps_s = ctx.enter_context(tc.tile_pool(name="ps_s", bufs=2, space="PSUM"))
        ps_o = ctx.enter_context(tc.tile_pool(name="ps_o", bufs=2, space="PSUM"))
        exp_pool = ctx.enter_context(tc.tile_pool(name="expp", bufs=6))
        norm_pool = ctx.enter_context(tc.tile_pool(name="normp", bufs=3))
        y_pool = ctx.enter_context(tc.tile_pool(name="yp", bufs=3))
        misc_sb = ctx.enter_context(tc.tile_pool(name="miscsb", bufs=1))

        wqkT_r = wqkT_d.rearrange("(co ci) f -> ci co f", ci=128)
        xT_r = xT_d.rearrange("(co ci) t -> ci co t", ci=128)
        wvT_r = wvT_d.rearrange("(co ci) f -> ci co f", ci=128)
        wpT_r = wpT_d.rearrange("(ko ki) n -> ki ko n", ki=128)

        # ---- PE warmup: dummy matmuls on zeroed SBUF fill the initial DMA
        # wait so the HAM clock gate reaches K=8/8 (2.4 GHz) before the first
        # real matmul; otherwise the first ~3.4us of real work runs at 1.2 GHz.
        wz = misc_sb.tile([128, 512], bf16, name="wz")
        nc.vector.memset(wz[:], 0.0)

        # single scratch psum tile reused by every dummy matmul (never read;
        # a fresh ring tile per dummy would alias the accumulating groups)
        psd = ps_misc.tile([128, 512], f32, tag="psb", name="warm")

        def warm_pe(n):
            # dummy matmuls on zeroed SBUF keep the HAM clock gate at K=8/8
            # (2.4 GHz) while the PE waits on DMA
            for _ in range(n):
                nc.tensor.matmul(psd[:], lhsT=wz[:, 0:128], rhs=wz[:],
                                 start=True, stop=True)

        warm_pe(7)

        # ---- DMA admission. The sync-engine HWDGE queue executes DMAs in
        # emission order, so order = priority. x1 goes on the scalar
        # engine's HWDGE queue, which runs in parallel with the sync queue
        # (baseline: x1/x2 arrived ~40/60us causing multi-us PE stalls and
        # HAM re-throttles). x2/x3 ride the sync queue after the weights.
        fbs = [4, 0, 5, 1, 6, 2, 7, 3]
        wtiles = {}
        wt = wqk_pool.tile([128, 8, 128], bf16, tag="wqk", name="wt4")
        nc.sync.dma_start(wt[:], wqkT_r[:, :, 4 * 128:5 * 128])
        wtiles[4] = wt
        xchunks = {}
        for tci in range(4):
            xchunks[tci] = xt_pool.tile([128, 8, 512], bf16, tag="xt",
                                        name=f"xc{tci}")
        # x0 split across BOTH queues as two half-merged transfers: the
        # halves land ~3us sooner than a per-co trickle (fewer descriptor
        # round-trips) and ~6us sooner than one 1MB descriptor
        d0 = nc.sync.dma_start(xchunks[0][:, 0:4, :], xT_r[:, 0:4, 0:512])
        nc.scalar.dma_start(xchunks[0][:, 4:8, :], xT_r[:, 4:8, 0:512])
        prev = d0

        # scalar-engine queue continues: wvT then x1 (in-order behind x0's
        # odd chunks, so they don't steal HBM bandwidth from x0; wvT is
        # needed ~17us in — far too late when queued on sync behind the
        # weights)
        for m in range(2):
            nc.scalar.dma_start(sb_mask2[:, m, :], mk_d[:])

        def _stage_w(fb, prev):
            wt = wqk_pool.tile([128, 8, 128], bf16, tag="wqk", name=f"wt{fb}")
            d = nc.sync.dma_start(wt[:], wqkT_r[:, :, fb * 128:(fb + 1) * 128])
            add_dep_helper(d.ins, prev.ins, sync=False, reason="stage w")
            wtiles[fb] = wt
            return d

        wv_dma = nc.scalar.dma_start(sb_wvT[:], wvT_r[:])

        prev = _stage_w(0, prev)
        for fb in (5, 1, 6, 2, 7, 3):
            prev = _stage_w(fb, prev)
        wv_last = prev

        # ones column of sb_v via exp(0)=1 — also warms the ACT exp table
        zeros = misc_sb.tile([128, 128], f32, name="zeros")
        nc.vector.memset(zeros[:], 0.0)
        ones_row = misc_sb.tile([1, 128], bf16, name="ones_row")
        nc.vector.memset(ones_row[:], 1.0)
        nc.scalar.activation(
            v_view[:, :, :, 64:65],
            zeros[:].rearrange("p (a b c) -> p a b c", a=16, b=8),
            Exp,
        )

        # ---- projection group emitters ----
        round_copy = {}

        def qk_group(tci, fb, pad=False):
            dst, pblk = (sb_kT, fb - 4) if fb >= 4 else (sb_qT, fb)
            ps = ps_misc.tile([128, 512], f32, tag="psb", name="psqk")
            for co in range(8):
                nc.tensor.matmul(
                    ps[:],
                    lhsT=wtiles[fb][:, co, :],
                    rhs=xchunks[tci][:, co, :],
                    start=(co == 0), stop=(co == 7),
                )
                if pad and co < 7:
                    # dummy between co-steps: x0 arrives per-co slower than
                    # the matmuls consume it; keeps HAM warm through the
                    # trickle instead of oscillating to K=4/8
                    warm_pe(1)
            cp = nc.vector.tensor_copy(
                dst[:, pblk, tci * 512:(tci + 1) * 512], ps[:])
            if fb == 4:
                round_copy[tci] = cp

        def v_group(tci, tb):
            tblk = tci * 4 + tb
            ps = ps_misc.tile([128, 512], f32, tag="psb", name="psv")
            for co in range(8):
                nc.tensor.matmul(
                    ps[:],
                    lhsT=xchunks[tci][:, co, tb * 128:(tb + 1) * 128],
                    rhs=sb_wvT[:, co, :],
                    start=(co == 0), stop=(co == 7),
                )
            nc.vector.tensor_copy(
                v_view[:, tblk, :, 0:64],
                ps[:].rearrange("p (h d) -> p h d", d=64),
            )

        def groups_for(tci):
            gs = []
            for fb in fbs:
                gs.append((tci, lambda t=tci, f=fb: qk_group(t, f)))
            for tb in range(4):
                gs.append((tci, lambda t=tci, b=tb: v_group(t, b)))
            return gs

        # ---- attention emitters ----
        def emit_proj_half(tblk, n, on_act=False):
            ysb = y_pool.tile([128, 512], bf16, tag="ysb", name="ysb")
            pj = ps_misc.tile([128, 512], f32, tag="psb", name="pj")
            for ko in range(4):
                nc.tensor.matmul(
                    pj[:],
                    lhsT=sb_attnT[:, ko, tblk * 128:(tblk + 1) * 128],
                    rhs=sb_wpT[:, ko, n * 512:(n + 1) * 512],
                    start=(ko == 0), stop=(ko == 3),
                )
            # tail projs alternate ACT/DVE so neither engine's copy queue
            # paces the PE
            if on_act and n == 0:
                nc.scalar.activation(ysb[:], pj[:], Copy)
            else:
                nc.vector.tensor_copy(ysb[:], pj[:])
            nc.sync.dma_start(
                y_d[tblk * 128:(tblk + 1) * 128, n * 512:(n + 1) * 512],
                ysb[:])

        def emit_proj(tblk, on_act=False):
            for n in range(2):
                emit_proj_half(tblk, n, on_act=on_act)

        def norm_store(po, rr, p_, qc, on_act=False, on_pe=False):
            att_slice = sb_attnT[rr:rr + 64, p_, qc * 512:(qc + 1) * 512]
            if on_pe:
                # tail path: head B's copies on ACT in parallel with head
                # A's DVE chain; head A's multiply reads its po straight
                # from PSUM (no att copy — same partitions, rr=0); bf16
                # broadcast on the now-idle PE (the gpsimd broadcast is
                # ~1-2us; fp32 matmul is a slow LOW/HIGH double pass).
                sums = norm_pool.tile([1, 512], f32, tag="sums", name="sums")
                if on_act:
                    nc.scalar.activation(att_slice, po[0:64, :], Copy)
                    nc.scalar.activation(sums[:], po[64:65, :], Copy)
                else:
                    nc.vector.tensor_copy(att_slice, po[0:64, :])
                    nc.vector.tensor_copy(sums[:], po[64:65, :])
                recip = norm_pool.tile([1, 512], f32, tag="recip",
                                       name="recip")
                nc.vector.reciprocal_approx_fast(out=recip[:], in_=sums[:])
                recipb = norm_pool.tile([1, 512], bf16, tag="recipb",
                                        name="recipb")
                nc.vector.tensor_copy(recipb[:], recip[:])
                # bc from ps_misc: a ps_o tile here would cycle into po's
                # slot and deadlock (bc waits po's readers; the mul reading
                # po would wait bc)
                bc = ps_misc.tile([128, 512], f32, tag="psb", name="bcps")
                nc.tensor.matmul(bc[:], lhsT=ones_row[:], rhs=recipb[:],
                                 start=True, stop=True)
                nc.vector.tensor_mul(att_slice, att_slice, bc[rr:rr + 64, :])
                return None
            sums = norm_pool.tile([1, 512], f32, tag="sums", name="sums")
            if on_act:
                nc.scalar.activation(att_slice, po[0:64, :], Copy)
                nc.scalar.activation(sums[:], po[64:65, :], Copy)
            else:
                nc.vector.tensor_copy(att_slice, po[0:64, :])
                nc.vector.tensor_copy(sums[:], po[64:65, :])
            recip = norm_pool.tile([1, 512], f32, tag="recip", name="recip")
            nc.vector.reciprocal_approx_fast(out=recip[:], in_=sums[:])
            # The broadcast-gated multiply is returned as a deferred
            # closure: emitted into the DVE FIFO only after the next
            # duo's filler casts, so its gpsimd-wait can't head-of-line
            # block the casts that free the fillers' PSUM slots.
            bcast = norm_pool.tile([128, 512], f32, tag="bcast",
                                   name="bcast")
            nc.gpsimd.partition_broadcast(bcast[:], recip[:])

            def _mul():
                nc.vector.tensor_mul(att_slice, att_slice,
                                     bcast[rr:rr + 64, :])
            return _mul

        def attn_duo(qc, di):
            """Generator: yields at PE-filler points."""
            hA, hB = 2 * di, 2 * di + 1
            nblk = 4 * qc + 4
            poA = ps_o.tile([65, 512], f32, tag="pso", name="poA")
            poB = ps_o.tile([65, 512], f32, tag="pso", name="poB")
            ets = {}

            def emit_pv(j):
                et = ets.pop(j)
                lo = max(0, (j - 4 * qc)) * 128
                for idx, (h, po) in enumerate(((hA, poA), (hB, poB))):
                    nc.tensor.matmul(
                        po[:, lo:512],
                        lhsT=v_view[:, j, h, :],
                        rhs=et[:, idx, lo:512],
                        start=(j == 0), stop=(j == nblk - 1),
                    )

            for j in range(nblk):
                lo = max(0, (j - 4 * qc)) * 128
                pss = ps_s.tile([128, 2, 512], f32, tag="pss", name="pss")
                for idx, rr in enumerate((0, 64)):
                    nc.tensor.matmul(
                        pss[:, idx, lo:512],
                        lhsT=sb_kT[rr:rr + 64, di, j * 128:(j + 1) * 128],
                        rhs=sb_qT[rr:rr + 64, di,
                                  qc * 512 + lo:(qc + 1) * 512],
                        start=True, stop=True,
                        tile_position=(rr, 0),
                    )
                et = exp_pool.tile([128, 2, 512], bf16, tag="expT", name="et")
                nc.scalar.activation(et[:, :, lo:512], pss[:, :, lo:512],
                                     Exp, scale=0.125)
                if j >= 4 * qc:
                    nc.vector.tensor_mul(et[:, :, lo:lo + 128],
                                         et[:, :, lo:lo + 128], sb_mask2[:])
                ets[j] = et
                if j >= 1:
                    yield
                    emit_pv(j - 1)
                else:
                    yield
            emit_pv(nblk - 1)
            last = qc == 3 and di == 3
            if last:
                # keep the HAM clock gate warm through the final norm's DVE
                # chain so the tail projections run at 2.4 GHz. Dedicated
                # tail-local psum tile: writing the head's psd here would
                # add a backward dep poisoning every ps_misc slot reuse.
                psd2 = ps_o.tile([128, 512], f32, tag="pso", name="warmtail")
                for _ in range(6):
                    nc.tensor.matmul(psd2[:], lhsT=wz[:, 0:128],
                                     rhs=et[:, 0, :], start=True, stop=True)
            # early qc: ACT has slack (small exp load), so po copies go
            # there; late qc is ACT-bound so they stay on DVE
            for m in (norm_store(poA, 0, di, qc, on_act=(qc <= 1),
                                 on_pe=last),
                      norm_store(poB, 64, di, qc, on_act=(qc <= 1) or last,
                                 on_pe=last)):
                if m is not None:
                    pend_muls.append(m)
            yield

        # ---- fused schedule ----
        # wpT / mask admission chained behind round-2 projection traffic
        def admit_late():
            prev = wv_last
            dma = nc.sync.dma_start(xchunks[1][:], xT_r[:, :, 512:1024])
            add_dep_helper(dma.ins, prev.ins, sync=False,
                           reason="stage x1 after weights")
            prev = dma
            dma = nc.sync.dma_start(sb_wpT[:], wpT_r[:])
            add_dep_helper(dma.ins, prev.ins, sync=False,
                           reason="admit wpT after weights")
            prev = dma
            for tci in (2, 3):
                dma = nc.sync.dma_start(
                    xchunks[tci][:], xT_r[:, :, tci * 512:(tci + 1) * 512])
                add_dep_helper(dma.ins, prev.ins, sync=False,
                               reason="stage late x")
                prev = dma

        # tci0 minimal prefix: exactly what attention (qc0, duo0) needs —
        # kT pair 0 (fb4), qT pair 0 (fb0). The remaining tci0 groups go to
        # the filler queue so duo d's needs (fb 4+d, fb d) and the v blocks
        # cascade in as earlier duos run (matching DMA arrival order).
        qk_group(0, 4, pad=True)
        qk_group(0, 0)
        admit_late()

        # Just-in-time fillers: projection group tci=k is emitted inside the
        # attention stretch qc=k, whose exp load it naturally balances
        # (PE attn(qc)+groups(tci=qc) ~ ACT exp(qc) for every qc). Each duo
        # pulls its own q/k feature blocks at start, prefetches the next
        # duo's mid-unit, and the first duo of each qc pulls the v blocks.
        gq = {}
        gv = {}
        for tci in range(4):
            for fb in fbs:
                if not (tci == 0 and fb in (4, 0)):
                    gq[(tci, fb)] = (lambda t=tci, f=fb: qk_group(t, f))
            for tb in range(4):
                gv[(tci, tb)] = (lambda t=tci, b=tb: v_group(t, b))

        def pop_pair(qc, di):
            if qc > 3:
                return
            for fb in (di, 4 + di):
                g = gq.pop((qc, fb), None)
                if g:
                    g()

        def pop_v(qc, tb):
            g = gv.pop((qc, tb), None)
            if g:
                g()

        pend_muls = []

        def flush_muls():
            for m in pend_muls:
                m()
            pend_muls.clear()

        # ---- qc=0: DMA-arrival-paced pops (delicate head ordering) ----
        qc = 0
        for di in range(4):
            pop_pair(qc, di)
            tick = 0
            for _ in attn_duo(qc, di):
                tick += 1
                if di == 0 and tick <= 4:
                    pop_v(qc, tick - 1)
                if tick == 1 and di < 3:
                    pop_pair(qc, di + 1)
                if tick == 2 and pend_muls:
                    flush_muls()

        # ---- qc>=1: one evenly-paced work queue per stretch. v units
        # first (PV j=4qc+tb inside duo di=0 consumes them), then qk and
        # proj halves interleaved; one unit per tick keeps the PE fed
        # through the diagonal (narrow) j-blocks where attention alone
        # can't cover the exp latency. ----
        for qc in range(1, 4):
            vs = [gv.pop((qc, tb)) for tb in range(4)]
            qks = [gq.pop((qc, fb)) for fb in fbs if (qc, fb) in gq]
            projs = [(lambda b=(qc - 1) * 4 + t, nn=n:
                      emit_proj_half(b, nn))
                     for t in range(4) for n in range(2)]
            rest = [u for pair in zip(qks, projs) for u in pair]
            rest += qks[len(projs):] + projs[len(qks):]
            total_ticks = 4 * (4 * qc + 5)
            rest_ticks = total_ticks - 4
            popped = 0
            tg = 0
            for di in range(4):
                tick = 0
                for _ in attn_duo(qc, di):
                    tg += 1
                    tick += 1
                    if tick == 2 and pend_muls:
                        flush_muls()
                    if tg <= 4:
                        vs[tg - 1]()
                    elif popped < len(rest) and (
                            (tg - 4) * len(rest) >= (popped + 1) * rest_ticks
                            or len(rest) - popped >= total_ticks - tg):
                        rest[popped]()
                        popped += 1
            while popped < len(rest):
                rest[popped]()
                popped += 1
        flush_muls()
        for tblk in range(12, 16):
            emit_proj(tblk, on_act=True)

    nc.compile()
    return nc


def _get_module():
    if "nc" not in _CACHE:
        _CACHE["nc"] = _build_module()
    return _CACHE["nc"]


def _make_trimask():
    # trimask[kk, q] = 1 iff q >= kk (diagonal 128x128 block)
    q = np.arange(128)[None, :]
    kk = np.arange(128)[:, None]
    return (q >= kk).astype(np.float32)


def make_in_maps(x, W_qkv, W_proj):
    import ml_dtypes

    bf16 = ml_dtypes.bfloat16
    x = np.asarray(x, dtype=np.float32)
    W_qkv = np.asarray(W_qkv, dtype=np.float32)
    W_proj = np.asarray(W_proj, dtype=np.float32)
    trimask = _make_trimask().astype(bf16)
    in_maps = []
    for c in range(N_CORES):
        b, g = c // 2, c % 2
        s = 512 * g
        wqk = np.concatenate([W_qkv[s:s + 512], W_qkv[1024 + s:1024 + s + 512]], 0)
        in_maps.append({
            "xT": np.ascontiguousarray(x[b].T).astype(bf16),
            "wqkT": np.ascontiguousarray(wqk.T).astype(bf16),
            "wvT": np.ascontiguousarray(W_qkv[2048 + s:2048 + s + 512].T).astype(bf16),
            "wpT": np.ascontiguousarray(W_proj[:, s:s + 512].T).astype(bf16),
            "trimask": trimask,
        })
    return in_maps


def run(x, W_qkv, W_proj, trace=False):
    """Returns (y_full [4,2048,1024], BassKernelResults)."""
    from concourse import bass_utils

    nc = _get_module()
    in_maps = make_in_maps(x, W_qkv, W_proj)
    res = bass_utils.run_bass_kernel_spmd(
        nc, in_maps, core_ids=list(range(N_CORES)), trace=trace)
    y = np.zeros((4, T, 1024), np.float32)
    for b in range(4):
        y[b] = (res.results[2 * b]["y"].astype(np.float32)
                + res.results[2 * b + 1]["y"].astype(np.float32))
    return y, res


def kernel(x, W_qkv, W_proj):
    y, _ = run(x, W_qkv, W_proj, trace=False)
    return y



# revision 87
# speedup vs baseline: 1.0521x; 1.0194x over previous
"""Causal self-attention (B=4, T=2048, C=1024, H=16, D=64) on 8 TRN2 NeuronCores.

Sharding: 4 batches x 2 head-groups (8 heads each). Core c handles batch c//2,
heads 8*(c%2) .. 8*(c%2)+7. Host pre-transposes x and slices/transposes the
weights so the device kernel needs no on-chip transposes. All matmul operands
are bf16 (fp32 PSUM accumulation); the y partials return as bf16.

Fused single-pass schedule: the QKV projection (4 t-chunks x 12 matmul groups)
and the attention/proj work are interleaved in one stream.

  projection: qkT[feat, T] = Wqk_g @ x^T, V[t, vfeat] = x @ Wv_g^T (+ ones col)
  attention:  per head-pair (2p, 2p+1) and query chunk qc: for each k-block j,
              S^T blocks for BOTH heads land in one [128,2,512] PSUM tile via
              row-tiled matmuls (concurrent on PE tiles (0,0)/(64,0)); ONE exp
              covers the pair. Scores are emitted in back-to-back j-pairs with
              the PVs batched two behind: tiled LDWEIGHTS can't use the PE's
              background weight buffer, so every entry/exit of a row-tiled
              pair exposes ~95ns — pairing halves that tax and lets PV->PV
              chains stream clean.
              P^T = exp(S^T/8) * causal masks, out^T[d|sum, q] = [V_h|1]^T P^T,
              normalized by approx-reciprocal + gpsimd partition-broadcast
              (deferred into the next duo's DVE stream to avoid head-of-line
              blocking the filler casts; the last duo instead broadcasts via a
              bf16 ones-matmul on the then-idle PE).
  proj:       y = attn^T.T @ Wp_g^T

Scheduling notes (measured on HW):
  - HAM clock gate: PE runs 1.2 GHz until ~3.4us of sustained activity, so
    dummy matmuls on zeroed SBUF warm it during the initial DMA wait and
    through the tail norm latency.
  - DMA: two HWDGE queues (sync + scalar engine) run in parallel; emission
    order == queue order == priority. x0 is split across both queues as two
    half-merged descriptors; wvT/masks ride the scalar queue; x1-x3/wpT
    trail the weights on sync.
  - qc>=1 fillers (qk groups, v groups, proj halves) are drained one per
    tick from a per-stretch queue (v first - same-stretch PV deadline),
    which keeps the PE fed through the narrow diagonal j-blocks where the
    ACT exp is the pacer.

Each core returns a [2048, 1024] bf16 partial; the host sums the two
head-group partials per batch in f32.
"""

import numpy as np

T = 2048
N_CORES = 8

_CACHE = {}


def _build_module():
    from collections import deque
    from contextlib import ExitStack

    import concourse.tile as tile
    from concourse.tile_rust import add_dep_helper
    from concourse import bacc, mybir

    f32 = mybir.dt.float32
    bf16 = mybir.dt.bfloat16
    Exp = mybir.ActivationFunctionType.Exp
    Copy = mybir.ActivationFunctionType.Copy

    nc = bacc.Bacc("TRN2", target_bir_lowering=False, debug=False,
                   num_devices=N_CORES)

    xT_d = nc.dram_tensor("xT", (1024, 2048), bf16, kind="ExternalInput").ap()
    wqkT_d = nc.dram_tensor("wqkT", (1024, 1024), bf16, kind="ExternalInput").ap()
    wvT_d = nc.dram_tensor("wvT", (1024, 512), bf16, kind="ExternalInput").ap()
    wpT_d = nc.dram_tensor("wpT", (512, 1024), bf16, kind="ExternalInput").ap()
    mk_d = nc.dram_tensor("trimask", (128, 128), bf16, kind="ExternalInput").ap()
    y_d = nc.dram_tensor("y", (2048, 1024), bf16, kind="ExternalOutput").ap()

    with tile.TileContext(nc) as tc, ExitStack() as ctx:
        pers = ctx.enter_context(tc.tile_pool(name="pers", bufs=1))
        sb_qT = pers.tile([128, 4, 2048], bf16, name="sb_qT")
        sb_kT = pers.tile([128, 4, 2048], bf16, name="sb_kT")
        sb_v = pers.tile([128, 16, 520], bf16, name="sb_v")
        v_view = sb_v[:].rearrange("p t (h e) -> p t h e", e=65)
        sb_attnT = pers.tile([128, 4, 2048], bf16, name="sb_attnT")
        sb_wpT = pers.tile([128, 4, 1024], bf16, name="sb_wpT")
        sb_mask2 = pers.tile([128, 2, 128], bf16, name="sb_mask2")
        sb_wvT = pers.tile([128, 8, 512], bf16, name="sb_wvT")

        wqk_pool = ctx.enter_context(tc.tile_pool(name="wqk", bufs=8))
        xt_pool = ctx.enter_context(tc.tile_pool(name="xt", bufs=4))
        ps_misc = ctx.enter_context(tc.tile_pool(name="ps_misc", bufs=2,
                                                 space="PSUM"))
        ps_s = ctx.enter_context(tc.tile_pool(name="ps_s", bufs=2, space="PSUM"))
        ps_o = ctx.enter_context(tc.tile_pool(name="ps_o", bufs=2, space="PSUM"))
        exp_pool = ctx.enter_context(tc.tile_pool(name="expp", bufs=6))
        norm_pool = ctx.enter_context(tc.tile_pool(name="normp", bufs=3))
        y_pool = ctx.enter_context(tc.tile_pool(name="yp", bufs=3))
        misc_sb = ctx.enter_context(tc.tile_pool(name="miscsb", bufs=1))

        wqkT_r = wqkT_d.rearrange("(co ci) f -> ci co f", ci=128)
        xT_r = xT_d.rearrange("(co ci) t -> ci co t", ci=128)
        wvT_r = wvT_d.rearrange("(co ci) f -> ci co f", ci=128)
        wpT_r = wpT_d.rearrange("(ko ki) n -> ki ko n", ki=128)

        # ---- PE warmup: dummy matmuls on zeroed SBUF fill the initial DMA
        # wait so the HAM clock gate reaches K=8/8 (2.4 GHz) before the first
        # real matmul; otherwise the first ~3.4us of real work runs at 1.2 GHz.
        wz = misc_sb.tile([128, 512], bf16, name="wz")
        nc.vector.memset(wz[:], 0.0)

        # single scratch psum tile reused by every dummy matmul (never read;
        # a fresh ring tile per dummy would alias the accumulating groups)
        psd = ps_misc.tile([128, 512], f32, tag="psb", name="warm")

        def warm_pe(n):
            # dummy matmuls on zeroed SBUF keep the HAM clock gate at K=8/8
            # (2.4 GHz) while the PE waits on DMA
            for _ in range(n):
                nc.tensor.matmul(psd[:], lhsT=wz[:, 0:128], rhs=wz[:],
                                 start=True, stop=True)

        warm_pe(10)

        # ---- DMA admission. The sync-engine HWDGE queue executes DMAs in
        # emission order, so order = priority. x1 goes on the scalar
        # engine's HWDGE queue, which runs in parallel with the sync queue
        # (baseline: x1/x2 arrived ~40/60us causing multi-us PE stalls and
        # HAM re-throttles). x2/x3 ride the sync queue after the weights.
        fbs = [4, 0, 5, 1, 6, 2, 7, 3]
        wtiles = {}
        wt = wqk_pool.tile([128, 8, 128], bf16, tag="wqk", name="wt4")
        nc.sync.dma_start(wt[:], wqkT_r[:, :, 4 * 128:5 * 128])
        wtiles[4] = wt
        xchunks = {}
        for tci in range(4):
            xchunks[tci] = xt_pool.tile([128, 8, 512], bf16, tag="xt",
                                        name=f"xc{tci}")
        # wt0 ahead of x0-lo: both prefix groups' weights land by ~9us and
        # the groups then pace purely on the x0 halves
        wt = wqk_pool.tile([128, 8, 128], bf16, tag="wqk", name="wt0")
        nc.sync.dma_start(wt[:], wqkT_r[:, :, 0:128])
        wtiles[0] = wt
        # x0 split across BOTH queues as two half-merged transfers: the
        # halves land ~3us sooner than a per-co trickle (fewer descriptor
        # round-trips) and ~6us sooner than one 1MB descriptor
        d0 = nc.sync.dma_start(xchunks[0][:, 0:4, :], xT_r[:, 0:4, 0:512])
        nc.scalar.dma_start(xchunks[0][:, 4:8, :], xT_r[:, 4:8, 0:512])
        prev = d0

        def _stage_w(fb, prev):
            wt = wqk_pool.tile([128, 8, 128], bf16, tag="wqk", name=f"wt{fb}")
            d = nc.sync.dma_start(wt[:], wqkT_r[:, :, fb * 128:(fb + 1) * 128])
            add_dep_helper(d.ins, prev.ins, sync=False, reason="stage w")
            wtiles[fb] = wt
            return d

        # The tile framework round-robins 8 HWDGE semaphores across BOTH
        # queues in EMISSION order; a slow transfer inside the first
        # 8-emission window blocks sem reuse for urgent DMAs 8 later. So:
        # the first 8 emissions are all small/urgent (wt4, wt0, x0lo, x0hi,
        # wt5, wt1, masks x2); wvT (1MB) is 9th — its sem is freed by the
        # first matmul's consumption of wt4.
        for fb in (5, 1):
            prev = _stage_w(fb, prev)
        for m in range(2):
            nc.scalar.dma_start(sb_mask2[:, m, :], mk_d[:])
        wv_dma = nc.scalar.dma_start(sb_wvT[:], wvT_r[:])

        for fb in (6, 2, 7, 3):
            prev = _stage_w(fb, prev)
        wv_last = prev

        # ones column of sb_v via exp(0)=1 — also warms the ACT exp table
        zeros = misc_sb.tile([128, 128], f32, name="zeros")
        nc.vector.memset(zeros[:], 0.0)
        ones_row = misc_sb.tile([1, 128], bf16, name="ones_row")
        nc.vector.memset(ones_row[:], 1.0)
        nc.scalar.activation(
            v_view[:, :, :, 64:65],
            zeros[:].rearrange("p (a b c) -> p a b c", a=16, b=8),
            Exp,
        )

        # ---- projection group emitters ----
        round_copy = {}

        def qk_group(tci, fb, pad=False):
            dst, pblk = (sb_kT, fb - 4) if fb >= 4 else (sb_qT, fb)
            ps = ps_misc.tile([128, 512], f32, tag="psb", name="psqk")
            for co in range(8):
                nc.tensor.matmul(
                    ps[:],
                    lhsT=wtiles[fb][:, co, :],
                    rhs=xchunks[tci][:, co, :],
                    start=(co == 0), stop=(co == 7),
                )
                if pad and co < 7:
                    # dummy between co-steps: x0 arrives per-co slower than
                    # the matmuls consume it; keeps HAM warm through the
                    # trickle instead of oscillating to K=4/8
                    warm_pe(1)
            cp = nc.vector.tensor_copy(
                dst[:, pblk, tci * 512:(tci + 1) * 512], ps[:])
            if fb == 4:
                round_copy[tci] = cp

        def v_group(tci, tb):
            tblk = tci * 4 + tb
            ps = ps_misc.tile([128, 512], f32, tag="psb", name="psv")
            for co in range(8):
                nc.tensor.matmul(
                    ps[:],
                    lhsT=xchunks[tci][:, co, tb * 128:(tb + 1) * 128],
                    rhs=sb_wvT[:, co, :],
                    start=(co == 0), stop=(co == 7),
                )
            nc.vector.tensor_copy(
                v_view[:, tblk, :, 0:64],
                ps[:].rearrange("p (h d) -> p h d", d=64),
            )

        def groups_for(tci):
            gs = []
            for fb in fbs:
                gs.append((tci, lambda t=tci, f=fb: qk_group(t, f)))
            for tb in range(4):
                gs.append((tci, lambda t=tci, b=tb: v_group(t, b)))
            return gs

        # ---- attention emitters ----
        def emit_proj_half(tblk, n, on_act=False):
            ysb = y_pool.tile([128, 512], bf16, tag="ysb", name="ysb")
            pj = ps_misc.tile([128, 512], f32, tag="psb", name="pj")
            for ko in range(4):
                nc.tensor.matmul(
                    pj[:],
                    lhsT=sb_attnT[:, ko, tblk * 128:(tblk + 1) * 128],
                    rhs=sb_wpT[:, ko, n * 512:(n + 1) * 512],
                    start=(ko == 0), stop=(ko == 3),
                )
            # tail projs alternate ACT/DVE so neither engine's copy queue
            # paces the PE
            if on_act and n == 0:
                nc.scalar.activation(ysb[:], pj[:], Copy)
            else:
                nc.vector.tensor_copy(ysb[:], pj[:])
            nc.sync.dma_start(
                y_d[tblk * 128:(tblk + 1) * 128, n * 512:(n + 1) * 512],
                ysb[:])

        def emit_proj(tblk, on_act=False):
            for n in range(2):
                emit_proj_half(tblk, n, on_act=on_act)

        def norm_store(po, rr, p_, qc, on_act=False, on_pe=False):
            att_slice = sb_attnT[rr:rr + 64, p_, qc * 512:(qc + 1) * 512]
            if on_pe:
                # tail path: head B's copies on ACT in parallel with head
                # A's DVE chain; head A's multiply reads its po straight
                # from PSUM (no att copy — same partitions, rr=0); bf16
                # broadcast on the now-idle PE (the gpsimd broadcast is
                # ~1-2us; fp32 matmul is a slow LOW/HIGH double pass).
                sums = norm_pool.tile([1, 512], f32, tag="sums", name="sums")
                if on_act:
                    nc.scalar.activation(att_slice, po[0:64, :], Copy)
                    nc.scalar.activation(sums[:], po[64:65, :], Copy)
                else:
                    nc.vector.tensor_copy(att_slice, po[0:64, :])
                    nc.vector.tensor_copy(sums[:], po[64:65, :])
                recip = norm_pool.tile([1, 512], f32, tag="recip",
                                       name="recip")
                nc.vector.reciprocal_approx_fast(out=recip[:], in_=sums[:])
                recipb = norm_pool.tile([1, 512], bf16, tag="recipb",
                                        name="recipb")
                nc.vector.tensor_copy(recipb[:], recip[:])
                # bc from ps_misc: a ps_o tile here would cycle into po's
                # slot and deadlock (bc waits po's readers; the mul reading
                # po would wait bc)
                bc = ps_misc.tile([128, 512], f32, tag="psb", name="bcps")
                nc.tensor.matmul(bc[:], lhsT=ones_row[:], rhs=recipb[:],
                                 start=True, stop=True)
                nc.vector.tensor_mul(att_slice, att_slice, bc[rr:rr + 64, :])
                return None
            sums = norm_pool.tile([1, 512], f32, tag="sums", name="sums")
            if on_act:
                nc.scalar.activation(att_slice, po[0:64, :], Copy)
                nc.scalar.activation(sums[:], po[64:65, :], Copy)
            else:
                nc.vector.tensor_copy(att_slice, po[0:64, :])
                nc.vector.tensor_copy(sums[:], po[64:65, :])
            recip = norm_pool.tile([1, 512], f32, tag="recip", name="recip")
            nc.vector.reciprocal_approx_fast(out=recip[:], in_=sums[:])
            # The broadcast-gated multiply is returned as a deferred
            # closure: emitted into the DVE FIFO only after the next
            # duo's filler casts, so its gpsimd-wait can't head-of-line
            # block the casts that free the fillers' PSUM slots.
            bcast = norm_pool.tile([128, 512], f32, tag="bcast",
                                   name="bcast")
            nc.gpsimd.partition_broadcast(bcast[:], recip[:])

            def _mul():
                nc.vector.tensor_mul(att_slice, att_slice,
                                     bcast[rr:rr + 64, :])
            return _mul

        def attn_duo(qc, di):
            """Generator: yields at PE-filler points."""
            hA, hB = 2 * di, 2 * di + 1
            nblk = 4 * qc + 4
            poA = ps_o.tile([65, 512], f32, tag="pso", name="poA")
            poB = ps_o.tile([65, 512], f32, tag="pso", name="poB")
            ets = {}

            def emit_pv(j):
                et = ets.pop(j)
                lo = max(0, (j - 4 * qc)) * 128
                for idx, (h, po) in enumerate(((hA, poA), (hB, poB))):
                    nc.tensor.matmul(
                        po[:, lo:512],
                        lhsT=v_view[:, j, h, :],
                        rhs=et[:, idx, lo:512],
                        start=(j == 0), stop=(j == nblk - 1),
                    )

            lastet = [None]

            def emit_scores(j):
                lo = max(0, (j - 4 * qc)) * 128
                pss = ps_s.tile([128, 2, 512], f32, tag="pss", name="pss")
                for idx, rr in enumerate((0, 64)):
                    nc.tensor.matmul(
                        pss[:, idx, lo:512],
                        lhsT=sb_kT[rr:rr + 64, di, j * 128:(j + 1) * 128],
                        rhs=sb_qT[rr:rr + 64, di,
                                  qc * 512 + lo:(qc + 1) * 512],
                        start=True, stop=True,
                        tile_position=(rr, 0),
                    )
                et = exp_pool.tile([128, 2, 512], bf16, tag="expT", name="et")
                nc.scalar.activation(et[:, :, lo:512], pss[:, :, lo:512],
                                     Exp, scale=0.125)
                if j >= 4 * qc:
                    nc.vector.tensor_mul(et[:, :, lo:lo + 128],
                                         et[:, :, lo:lo + 128], sb_mask2[:])
                ets[j] = et
                lastet[0] = et

            # scores emitted in back-to-back pairs, PVs batched two behind:
            # every other tiled-LDW exposure (~95ns) is saved vs strict
            # S/PV alternation, and PV->PV chains stream clean
            for p in range(nblk // 2):
                emit_scores(2 * p)
                emit_scores(2 * p + 1)
                yield
                if p >= 1:
                    emit_pv(2 * p - 2)
                    emit_pv(2 * p - 1)
                yield
            emit_pv(nblk - 2)
            emit_pv(nblk - 1)
            last = qc == 3 and di == 3
            if last:
                # keep the HAM clock gate warm through the final norm's DVE
                # chain so the tail projections run at 2.4 GHz. Dedicated
                # tail-local psum tile: writing the head's psd here would
                # add a backward dep poisoning every ps_misc slot reuse.
                psd2 = ps_o.tile([128, 512], f32, tag="pso", name="warmtail")
                for _ in range(6):
                    nc.tensor.matmul(psd2[:], lhsT=wz[:, 0:128],
                                     rhs=lastet[0][:, 0, :],
                                     start=True, stop=True)
            # early qc: ACT has slack (small exp load), so po copies go
            # there; late qc is ACT-bound so they stay on DVE
            for m in (norm_store(poA, 0, di, qc, on_act=(qc <= 1),
                                 on_pe=last),
                      norm_store(poB, 64, di, qc, on_act=(qc <= 1) or last,
                                 on_pe=last)):
                if m is not None:
                    pend_muls.append(m)
            yield

        # ---- fused schedule ----
        # wpT / mask admission chained behind round-2 projection traffic
        def admit_late():
            prev = wv_last
            dma = nc.sync.dma_start(xchunks[1][:], xT_r[:, :, 512:1024])
            add_dep_helper(dma.ins, prev.ins, sync=False,
                           reason="stage x1 after weights")
            prev = dma
            dma = nc.sync.dma_start(sb_wpT[:], wpT_r[:])
            add_dep_helper(dma.ins, prev.ins, sync=False,
                           reason="admit wpT after weights")
            prev = dma
            for tci in (2, 3):
                dma = nc.sync.dma_start(
                    xchunks[tci][:], xT_r[:, :, tci * 512:(tci + 1) * 512])
                add_dep_helper(dma.ins, prev.ins, sync=False,
                               reason="stage late x")
                prev = dma

        # tci0 minimal prefix: exactly what attention (qc0, duo0) needs —
        # kT pair 0 (fb4), qT pair 0 (fb0). The remaining tci0 groups go to
        # the filler queue so duo d's needs (fb 4+d, fb d) and the v blocks
        # cascade in as earlier duos run (matching DMA arrival order).
        qk_group(0, 4, pad=True)
        qk_group(0, 0)
        admit_late()

        # Just-in-time fillers: projection group tci=k is emitted inside the
        # attention stretch qc=k, whose exp load it naturally balances
        # (PE attn(qc)+groups(tci=qc) ~ ACT exp(qc) for every qc). Each duo
        # pulls its own q/k feature blocks at start, prefetches the next
        # duo's mid-unit, and the first duo of each qc pulls the v blocks.
        gq = {}
        gv = {}
        for tci in range(4):
            for fb in fbs:
                if not (tci == 0 and fb in (4, 0)):
                    gq[(tci, fb)] = (lambda t=tci, f=fb: qk_group(t, f))
            for tb in range(4):
                gv[(tci, tb)] = (lambda t=tci, b=tb: v_group(t, b))

        def pop_pair(qc, di):
            if qc > 3:
                return
            for fb in (di, 4 + di):
                g = gq.pop((qc, fb), None)
                if g:
                    g()

        def pop_v(qc, tb):
            g = gv.pop((qc, tb), None)
            if g:
                g()

        pend_muls = []

        def flush_muls():
            for m in pend_muls:
                m()
            pend_muls.clear()

        # ---- qc=0: DMA-arrival-paced pops (delicate head ordering) ----
        qc = 0
        for di in range(4):
            pop_pair(qc, di)
            tick = 0
            for _ in attn_duo(qc, di):
                tick += 1
                if di == 0 and tick <= 4:
                    pop_v(qc, tick - 1)
                # next duo's weight groups pop LATE (tick 4): their wt
                # tiles land ~17.5us; popped at tick 1 they head-of-line
                # block the PE FIFO while ready attention work waits
                if tick == 4 and di < 3:
                    pop_pair(qc, di + 1)
                if tick == 2 and pend_muls:
                    flush_muls()

        # ---- qc>=1: one evenly-paced work queue per stretch. v units
        # first (PV j=4qc+tb inside duo di=0 consumes them), then qk and
        # proj halves interleaved; one unit per tick keeps the PE fed
        # through the diagonal (narrow) j-blocks where attention alone
        # can't cover the exp latency. ----
        for qc in range(1, 4):
            vs = [gv.pop((qc, tb)) for tb in range(4)]
            qks = [gq.pop((qc, fb)) for fb in fbs if (qc, fb) in gq]
            projs = [(lambda b=(qc - 1) * 4 + t, nn=n:
                      emit_proj_half(b, nn))
                     for t in range(4) for n in range(2)]
            rest = [u for pair in zip(qks, projs) for u in pair]
            rest += qks[len(projs):] + projs[len(qks):]
            total_ticks = 4 * (4 * qc + 5)
            rest_ticks = total_ticks - 4
            popped = 0
            tg = 0
            for di in range(4):
                tick = 0
                for _ in attn_duo(qc, di):
                    tg += 1
                    tick += 1
                    if tick == 2 and pend_muls:
                        flush_muls()
                    if tg <= 4:
                        vs[tg - 1]()
                    elif popped < len(rest) and (
                            (tg - 4) * len(rest) >= (popped + 1) * rest_ticks
                            or len(rest) - popped >= total_ticks - tg):
                        rest[popped]()
                        popped += 1
            while popped < len(rest):
                rest[popped]()
                popped += 1
        flush_muls()
        for tblk in range(12, 16):
            emit_proj(tblk, on_act=True)

    nc.compile()
    return nc


def _get_module():
    if "nc" not in _CACHE:
        _CACHE["nc"] = _build_module()
    return _CACHE["nc"]


def _make_trimask():
    # trimask[kk, q] = 1 iff q >= kk (diagonal 128x128 block)
    q = np.arange(128)[None, :]
    kk = np.arange(128)[:, None]
    return (q >= kk).astype(np.float32)


def make_in_maps(x, W_qkv, W_proj):
    import ml_dtypes

    bf16 = ml_dtypes.bfloat16
    x = np.asarray(x, dtype=np.float32)
    W_qkv = np.asarray(W_qkv, dtype=np.float32)
    W_proj = np.asarray(W_proj, dtype=np.float32)
    trimask = _make_trimask().astype(bf16)
    in_maps = []
    for c in range(N_CORES):
        b, g = c // 2, c % 2
        s = 512 * g
        wqk = np.concatenate([W_qkv[s:s + 512], W_qkv[1024 + s:1024 + s + 512]], 0)
        in_maps.append({
            "xT": np.ascontiguousarray(x[b].T).astype(bf16),
            "wqkT": np.ascontiguousarray(wqk.T).astype(bf16),
            "wvT": np.ascontiguousarray(W_qkv[2048 + s:2048 + s + 512].T).astype(bf16),
            "wpT": np.ascontiguousarray(W_proj[:, s:s + 512].T).astype(bf16),
            "trimask": trimask,
        })
    return in_maps


def run(x, W_qkv, W_proj, trace=False):
    """Returns (y_full [4,2048,1024], BassKernelResults)."""
    from concourse import bass_utils

    nc = _get_module()
    in_maps = make_in_maps(x, W_qkv, W_proj)
    res = bass_utils.run_bass_kernel_spmd(
        nc, in_maps, core_ids=list(range(N_CORES)), trace=trace)
    y = np.zeros((4, T, 1024), np.float32)
    for b in range(4):
        y[b] = (res.results[2 * b]["y"].astype(np.float32)
                + res.results[2 * b + 1]["y"].astype(np.float32))
    return y, res


def kernel(x, W_qkv, W_proj):
    y, _ = run(x, W_qkv, W_proj, trace=False)
    return y

